# revision 20
# baseline (speedup 1.0000x reference)
"""Trainium2 Bass kernel for nn_Autorec_DG_13116830122688 (AutoRec + GraphConv0D).

Math (reference):
    h   = sigmoid(x @ enc_w.T + enc_b)                      [N, 500]
    agg = segment_sum(h[src] * edge_weight, dst, N)
    hm  = conv_w * agg + (1 - conv_w) * h
    p   = clip(hm @ dec_w.T + dec_b, 1, 5)
    p   = where(ft_n0 == 0 rows, fill, p); where(ft_n1 == 0 cols, fill, p)

Strategy (8 NeuronCores, data-parallel over users):
  - Shard users 2500/core (padded to 2560 = 20x128 tiles).
  - x is transposed + cast to bf16 on the HOST into per-tile item-major
    blocks [20][128p][47k*128u], so the encoder is a pure matmul stream:
    one contiguous 1.5 MB DMA per user tile, 47 accumulating matmuls
    against SBUF-resident enc_w (bias folded in as an extra input col,
    hidden padded to 504 with zero weights so h cols 500-503 are defined
    -> no memset needed).  ACT sigmoid -> h bf16.
  - AllGather h (bf16) in 6 uneven chunks overlapped with the encoder;
    the last chunk is a single tile so the exposed tail is short.
  - Message passing: edges filtered (masked-dst dropped), scaled by
    conv_w, sorted by dst, packed into 128-edge blocks per 128-dst tile.
    Gather h[src] via indirect DMA; multiply with host-built sparse
    [128e x 128d] weight blocks on the PE: aggT += G.T @ W in PSUM,
    hidden-major, feeding the decoder without a transpose.  All of a
    tile's weight blocks arrive in ONE packed DMA.  Self-loops use a
    plain DMA of this core's own h rows + a diagonal weight block.
  - Decoder: p = hmT.T @ dec_w.T with column mask / fill baked into
    host-prepped weights plus bias & row-fill hidden units.  DVE clips
    to [1,5] writing fp16; one 1.5 MB output DMA per tile (fp16 out is
    upcast to f32 on the host).
  - DMA queue split: x-stream + gather-side prefetches on the sync ring,
    weights on the scalar ring, h/out writes + rv on the vector ring, so
    prefetches never sit behind compute-dependent writes.
"""

import os
import sys

import numpy as np

for _p in ("/opt/trn_rl_repo",):
    if _p not in sys.path and os.path.isdir(_p):
        sys.path.insert(0, _p)

import ml_dtypes  # noqa: E402

# ---- problem constants (hardcoded per contest rules) ----
N_USERS = 20000
N_ITEMS = 6000
HIDDEN = 500
M = 8  # cores
UPC = N_USERS // M  # 2500 users per core
UT = 20  # user tiles per core
UPAD = UT * 128  # 2560
KC = 47  # item chunks of 128 (6016 = 47*128 >= 6001 incl. bias col)
IPAD = KC * 128  # 6016
HPAD = 504  # hidden padded: 4 chunks of 126 (500 real + bias/mask units)
R_MIN, R_MAX = 1.0, 5.0
# all-gather chunk boundaries in user tiles (cumulative).  The chunked
# all-gathers execute serially on the CC core and compete with the
# encoder for HBM, so the chain must start early (small first chunk) and
# end with small chunks (short exposed tail after the encoder).
CC_TILE_BOUNDS = [5, 10, 14, 17, 20]
# sources in AG chunks 1..NEARLY_CHUNKS (user tiles < 17) are "early": their
# gather blocks only depend on the first NEARLY_CHUNKS all-gathers.
NEARLY_CHUNKS = 4

_bf16 = ml_dtypes.bfloat16
_f16 = np.float16

_PROGRAM_CACHE = {}


def _build_program(S, NE):
    """Build the SPMD Bass program. S = per-tile gather-block counts;
    NE = per-tile count of leading "early" blocks whose sources all live in
    AG chunks 1..NEARLY_CHUNKS."""
    import concourse.bass as bass
    import concourse.bacc as bacc
    import concourse.mybir as mybir
    from concourse.tile import TileContext

    P = 128
    f32 = mybir.dt.float32
    bf16 = mybir.dt.bfloat16
    f16 = mybir.dt.float16
    i32 = mybir.dt.int32
    NBLK = sum(S)
    SMAX = max(S)
    BOFF = [sum(S[:t]) for t in range(UT)]

    nc = bacc.Bacc(
        "TRN2",
        target_bir_lowering=False,
        debug=False,
        num_devices=M,
        num_swdge_queues=4,
    )

    x_d = nc.declare_dram_parameter("x", [UT, P, IPAD], bf16, isOutput=False)
    encw_d = nc.declare_dram_parameter("encw", [P, KC * HPAD], bf16, isOutput=False)
    decw_d = nc.declare_dram_parameter("decw", [P, 4 * N_ITEMS], bf16, isOutput=False)
    si_d = nc.declare_dram_parameter("sidx", [P, NBLK], i32, isOutput=False)
    wb_d = nc.declare_dram_parameter("wblk", [P, (NBLK + UT) * P], bf16, isOutput=False)
    rv_d = nc.declare_dram_parameter("rowvec", [2, UPAD], bf16, isOutput=False)
    out_d = nc.declare_dram_parameter("out", [UPC, N_ITEMS], f16, isOutput=True)

    h_loc = nc.dram_tensor("h_loc", [UPAD, HPAD], bf16)
    h_full = nc.dram_tensor("h_full", [M * UPAD, HPAD], bf16, addr_space="Shared")
    CC_BOUNDS = CC_TILE_BOUNDS

    with TileContext(nc) as tc:
        with (
            # tiles the PE streams from (moving + stationary sources that are
            # not DMA-hammered) live on the RIGHT side of SBUF; the DMA fire
            # hose (x stream, gathers, h/p staging) on the LEFT, to minimize
            # SBUF port contention with the PE's xbus reads.
            tc.tile_pool(name="const", bufs=1, side="right") as cpool,
            tc.tile_pool(name="hmt", bufs=2, side="right") as mpool,
            tc.tile_pool(name="wbl", bufs=3, side="right") as wpool,
            tc.tile_pool(name="xin", bufs=3, side="left") as xpool,
            tc.tile_pool(name="hsb", bufs=3, side="left") as hpool,
            tc.tile_pool(name="gat", bufs=2 * (SMAX + 1) + 2, side="left") as gpool,
            tc.tile_pool(name="pout", bufs=2, side="left") as opool,
            tc.tile_pool(name="ps_a", bufs=2, space="PSUM") as ps_a,
            tc.tile_pool(name="ps_d", bufs=6, space="PSUM") as ps_d,
        ):
            # weights on the scalar HWDGE ring, in pieces fine enough that
            # tile 0's matmul stream is never far behind the weight DMA.
            enc_sb = cpool.tile([P, KC * HPAD], bf16, tag="encw")
            for e0, e1 in ((0, 2), (2, 4), (4, 8), (8, 12), (12, 16), (16, 24),
                           (24, 32), (32, 40), (40, KC)):
                nc.scalar.dma_start(
                    out=enc_sb[:, e0 * HPAD : e1 * HPAD],
                    in_=encw_d[:, e0 * HPAD : e1 * HPAD],
                )
            si_sb = cpool.tile([P, NBLK], i32, tag="sidx")
            nc.scalar.dma_start(out=si_sb[:], in_=si_d[:])
            dec_sb = cpool.tile([P, 4 * N_ITEMS], bf16, tag="decw")
            nc.scalar.dma_start(out=dec_sb[:], in_=decw_d[:])

            # ---------------- Phase 1: encoder ----------------
            for ut in range(UT):
                xb = xpool.tile([P, IPAD], bf16, tag="xb")
                nc.sync.dma_start(out=xb[:], in_=x_d[ut])
                h_ps = ps_a.tile([P, 512], f32, tag="acc")
                for k in range(KC):
                    nc.tensor.matmul(
                        out=h_ps[:, :HPAD],
                        lhsT=xb[:, k * P : (k + 1) * P],
                        rhs=enc_sb[:, k * HPAD : (k + 1) * HPAD],
                        start=(k == 0),
                        stop=(k == KC - 1),
                    )
                hsb = hpool.tile([P, HPAD], bf16, tag="hsb")
                nc.scalar.activation(
                    out=hsb[:],
                    in_=h_ps[:, :HPAD],
                    func=mybir.ActivationFunctionType.Sigmoid,
                )
                nc.scalar.dma_start(
                    out=h_loc[ut * P : (ut + 1) * P, :], in_=hsb[:]
                )
                # ---- Phase 2 (interleaved): chunked all-gather ----
                if (ut + 1) in CC_BOUNDS:
                    j = CC_BOUNDS.index(ut + 1)
                    lo = 0 if j == 0 else CC_BOUNDS[j - 1]
                    hi = CC_BOUNDS[j]
                    off = M * lo * P
                    nc.gpsimd.collective_compute(
                        "AllGather",
                        mybir.AluOpType.bypass,
                        replica_groups=[list(range(M))],
                        ins=[h_loc[lo * P : hi * P, :]],
                        outs=[h_full[off : off + M * (hi - lo) * P, :]],
                    )

            # ---------------- Phase 3: message passing + decoder ----------------
            # Software-pipelined so the PE never idles (any PE stall resets a
            # ~3us half-speed ramp): gathers are issued two tiles ahead, and
            # tile t+1's message-passing matmuls + hmT copy run in the MIDDLE
            # of tile t's decoder so the agg->hmT copy latency is hidden.

            # blocks whose sources all land in AG chunks 1..NEARLY ("early"
            # blocks, host-packed first) can gather before the last all-
            # gathers complete: slicing h_full keeps the dependency off the
            # final AG chunks.
            EARLY_ROWS = M * CC_TILE_BOUNDS[NEARLY_CHUNKS - 1] * P

            def issue_early(t):
                gts = []
                for s in range(NE[t]):
                    b = BOFF[t] + s
                    gt = gpool.tile([P, HPAD], bf16, tag="gt", name=f"gt{t}_{s}")
                    nc.gpsimd.indirect_dma_start(
                        out=gt[:],
                        out_offset=None,
                        in_=h_full[0:EARLY_ROWS, :],
                        in_offset=bass.IndirectOffsetOnAxis(
                            ap=si_sb[:, b : b + 1], axis=0
                        ),
                    )
                    gts.append(gt)
                return gts

            def issue_late(t, gts):
                for s in range(NE[t], S[t]):
                    b = BOFF[t] + s
                    gt = gpool.tile([P, HPAD], bf16, tag="gt", name=f"gl{t}_{s}")
                    nc.gpsimd.indirect_dma_start(
                        out=gt[:],
                        out_offset=None,
                        in_=h_full[:],
                        in_offset=bass.IndirectOffsetOnAxis(
                            ap=si_sb[:, b : b + 1], axis=0
                        ),
                    )
                    gts.append(gt)
                # self-loop block: this core's own contiguous h rows via a
                # plain DMA and a diagonal weight block (no indirect gather).
                gself = gpool.tile([P, HPAD], bf16, tag="gt", name=f"gs{t}")
                nc.sync.dma_start(
                    out=gself[:], in_=h_loc[t * P : (t + 1) * P, :]
                )
                gts.append(gself)
                # all of this tile's weight blocks in one packed DMA
                nsb = S[t] + 1
                wsb = wpool.tile([P, (SMAX + 1) * P], bf16, tag="wb", name=f"wsb{t}")
                nc.sync.dma_start(
                    out=wsb[:, : nsb * P],
                    in_=wb_d[:, (BOFF[t] + t) * P : (BOFF[t] + t + nsb) * P],
                )
                return gts, wsb

            def issue_gathers(t):
                return issue_late(t, issue_early(t))

            def mp_compute(t, gts, wsb):
                nsb = S[t] + 1
                agg_ps = ps_a.tile([P, 512], f32, tag="acc", name=f"agg{t}")
                # keep each PSUM sub-region's accumulation group contiguous:
                # interleaved start=True matmuls in one bank clobber each
                # other's accumulation state.
                for c in range(4):
                    for s in range(nsb):
                        nc.tensor.matmul(
                            out=agg_ps[0:126, c * P : (c + 1) * P],
                            lhsT=gts[s][:, c * 126 : (c + 1) * 126],
                            rhs=wsb[:, s * P : (s + 1) * P],
                            start=(s == 0),
                            stop=(s == nsb - 1),
                        )
                hmT = mpool.tile([P, 512], bf16, tag="hmT", name=f"hmT{t}")
                nc.scalar.activation(
                    out=hmT[0:126, :],
                    in_=agg_ps[0:126, :],
                    func=mybir.ActivationFunctionType.Copy,
                )
                # hidden unit 500 (chunk 3, row 122): decoder-bias unit
                # hidden unit 501 (chunk 3, row 123): row-mask fill unit
                nc.scalar.dma_start(
                    out=hmT[122:124, 3 * P : 4 * P],
                    in_=rv_d[0:2, t * P : (t + 1) * P],
                )
                return hmT

            def dec_half(t, half, hmT, psb):
                # c-outer so 6 consecutive matmuls share the same stationary
                # operand (hmT chunk c) — saves the per-matmul weight-reload
                # chain when the compiler/hardware can exploit it.
                pps = []
                for nn in range(6):
                    pp = ps_d.tile(
                        [P, 512], f32, tag="pp", name=f"pp{t}_{half}_{nn}"
                    )
                    pps.append(pp)
                for c in range(4):
                    for nn in range(6):
                        n = half * 6 + nn
                        nc.tensor.matmul(
                            out=pps[nn][:, :500],
                            lhsT=hmT[0:126, c * P : (c + 1) * P],
                            rhs=dec_sb[0:126, c * N_ITEMS + n * 500 : c * N_ITEMS + (n + 1) * 500],
                            start=(c == 0),
                            stop=(c == 3),
                        )
                for nn in range(6):
                    n = half * 6 + nn
                    nc.vector.tensor_scalar(
                        out=psb[:, n * 500 : (n + 1) * 500],
                        in0=pps[nn][:, :500],
                        scalar1=R_MAX,
                        scalar2=R_MIN,
                        op0=mybir.AluOpType.min,
                        op1=mybir.AluOpType.max,
                    )

            # pipeline head: all early gathers for tiles 0/1 are issued before
            # any late gather so the gpsimd queue never stalls on the final
            # all-gathers while early work remains.
            e0 = issue_early(0)
            e1 = issue_early(1)
            pend = {0: issue_late(0, e0), 1: issue_late(1, e1)}
            hmTs = {0: mp_compute(0, *pend.pop(0))}
            for t in range(UT):
                hmT = hmTs.pop(t)
                psb = opool.tile([P, N_ITEMS], f16, tag="psb", name=f"psb{t}")
                nu = UPC - t * P if t == UT - 1 else P  # 68 on the last tile
                dec_half(t, 0, hmT, psb)
                if t + 2 < UT:
                    pend[t + 2] = issue_gathers(t + 2)
                if t + 1 < UT:
                    hmTs[t + 1] = mp_compute(t + 1, *pend.pop(t + 1))
                nc.scalar.dma_start(
                    out=out_d[t * P : t * P + nu, 0:3000],
                    in_=psb[:nu, 0:3000],
                )
                dec_half(t, 1, hmT, psb)
                nc.scalar.dma_start(
                    out=out_d[t * P : t * P + nu, 3000:6000],
                    in_=psb[:nu, 3000:6000],
                )

    nc.finalize()
    return nc


def _prep_host(x, edge_index, edge_weight, ft_n0, ft_n1, fill_const,
               enc_w, enc_b, dec_w, dec_b, conv_w):
    """All host-side preprocessing: sharding, weight prep, edge packing."""
    x = np.asarray(x, np.float32)
    src = np.asarray(edge_index[0], np.int64)
    dst = np.asarray(edge_index[1], np.int64)
    w = np.asarray(edge_weight, np.float32)
    ft_n0 = np.asarray(ft_n0)
    ft_n1 = np.asarray(ft_n1)
    fill = float(np.asarray(fill_const))
    conv = float(np.asarray(conv_w))
    enc_w = np.asarray(enc_w, np.float32)
    enc_b = np.asarray(enc_b, np.float32)
    dec_w = np.asarray(dec_w, np.float32)
    dec_b = np.asarray(dec_b, np.float32)

    rowmask = ft_n0 == 0  # rows forced to fill
    colmask = ft_n1 == 0  # cols forced to fill

    # ---- x per core: item-major per-tile blocks [UT, 128p, KC*128u] bf16 ----
    # x_host[c][t][p][k*128+j] = xpad[c, t*128+j, k*128+p]
    xp = np.zeros((M, UPAD, IPAD), np.float32)
    xp[:, :UPC, :N_ITEMS] = x.reshape(M, UPC, N_ITEMS)
    xp[:, :, N_ITEMS] = 1.0  # encoder-bias input column
    xpb = xp.astype(_bf16)
    del xp
    x_host = np.ascontiguousarray(
        xpb.reshape(M, UT, 128, KC, 128).transpose(0, 1, 4, 3, 2)
    ).reshape(M, UT, 128, IPAD)
    del xpb

    # ---- encoder weights: [6016, 504] -> [128, 47*504] chunk-major ----
    ewp = np.zeros((IPAD, HPAD), np.float32)
    ewp[:N_ITEMS, :HIDDEN] = enc_w.T
    ewp[N_ITEMS, :HIDDEN] = enc_b
    enc_host = np.ascontiguousarray(
        ewp.reshape(KC, 128, HPAD).transpose(1, 0, 2).reshape(128, KC * HPAD)
    ).astype(_bf16)

    # ---- decoder weights with baked column mask / bias / fill units ----
    dw = dec_w.T.copy()  # [500, 6000]
    dw[:, colmask] = 0.0
    hp = np.zeros((HPAD, N_ITEMS), np.float32)
    hp[:HIDDEN] = dw
    hp[HIDDEN] = np.where(colmask, fill, dec_b)  # bias unit
    hp[HIDDEN + 1] = fill  # row-mask fill unit (all cols)
    dec_host = np.zeros((128, 4, N_ITEMS), np.float32)
    dec_host[:126] = hp.reshape(4, 126, N_ITEMS).transpose(1, 0, 2)
    dec_host = np.ascontiguousarray(dec_host.reshape(128, 4 * N_ITEMS)).astype(_bf16)

    # ---- edges: filter masked dst, fold conv_w, sort by dst ----
    keep = ~rowmask[dst]
    src_a = src[keep]
    dst_a = dst[keep]
    w_a = w[keep] * conv

    order = np.argsort(dst_a, kind="stable")
    src_a, dst_a, w_a = src_a[order], dst_a[order], w_a[order]

    core = dst_a // UPC
    ldst = dst_a - core * UPC
    tile_g = core * UT + ldst // 128  # global tile id (sorted ascending)
    din = (ldst % 128).astype(np.int64)
    counts = np.bincount(tile_g, minlength=M * UT).reshape(M, UT)

    # gather index into the PADDED all-gathered h table.
    # h_full layout after the uneven chunked all-gather: chunk j covers local
    # rows [lo_j*128, hi_j*128) of every core, concatenated core-major:
    # row = off_j + core * crows_j + (local - lo_j*128)
    src_core = src_a // UPC
    src_loc = src_a % UPC
    bounds_rows = np.array([b * 128 for b in CC_TILE_BOUNDS])
    starts_rows = np.concatenate([[0], bounds_rows[:-1]])
    crows = bounds_rows - starts_rows
    offs = np.concatenate([[0], np.cumsum(M * crows)[:-1]])
    cjs = np.searchsorted(bounds_rows, src_loc, side="right")
    gsrc_e = (
        offs[cjs] + src_core * crows[cjs] + (src_loc - starts_rows[cjs])
    ).astype(np.int64)

    # per-TILE block quota (max over cores, so the SPMD program is identical
    # on every core) instead of one global maximum: ~15-20% fewer gathers.
    S_t = np.maximum(1, np.ceil(counts.max(axis=0) / 128).astype(np.int64))
    boff = np.concatenate([[0], np.cumsum(S_t)[:-1]])
    NBLK = int(S_t.sum())

    # per-tile packed layout: tile t's blocks at columns
    # [(boff[t]+t)*128, (boff[t]+t+S_t+1)*128); gathers first, self last.
    # Within each tile, edges whose source is in AG chunks 1..NEARLY_CHUNKS
    # ("early") are packed into the leading blocks; NE[t] = number of blocks
    # (uniform across cores) that contain only early edges.
    EARLY_LOC = CC_TILE_BOUNDS[NEARLY_CHUNKS - 1] * 128
    si_host = np.zeros((M, 128, NBLK), np.int32)
    wblk_host = np.zeros((M, 128, (NBLK + UT) * 128), np.float32)
    first_late_blk = np.full((M, UT), 10**6, np.int64)
    starts = np.zeros(M * UT + 1, np.int64)
    np.cumsum(counts.reshape(-1), out=starts[1:])
    for g in range(M * UT):
        c, t = divmod(g, UT)
        n = int(counts[c, t])
        sl = slice(starts[g], starts[g] + n)
        cap = int(S_t[t]) * 128
        late = src_loc[sl] >= EARLY_LOC
        order2 = np.argsort(late, kind="stable")  # early edges first
        gi = np.zeros(cap, np.int64)
        wi = np.zeros(cap, np.float32)
        di = np.zeros(cap, np.int64)
        gi[:n] = gsrc_e[sl][order2]
        wi[:n] = w_a[sl][order2]
        di[:n] = din[sl][order2]
        n_late = int(late.sum())
        first_late_blk[c, t] = (n - n_late) // 128 if n_late else int(S_t[t])
        b0 = int(boff[t])
        base = (b0 + t) * 128
        for q in range(int(S_t[t])):
            blk = slice(q * 128, (q + 1) * 128)
            si_host[c, :, b0 + q] = gi[blk]
            wblk_host[c, np.arange(128), base + q * 128 + di[blk]] = wi[blk]
    NE = tuple(
        int(min(first_late_blk[:, t].min(), S_t[t])) for t in range(UT)
    )
    # diagonal self-loop weight blocks at slot S_t[t] of each tile:
    # applies (1-conv)*live(d) to h_loc rows of tile t.
    lv = np.zeros((M, UPAD), np.float32)
    lv[:, :UPC] = (~rowmask).reshape(M, UPC).astype(np.float32) * (1.0 - conv)
    di128 = np.arange(128)
    for t in range(UT):
        base = (int(boff[t]) + t + int(S_t[t])) * 128
        wblk_host[:, di128, base + di128] = lv[:, t * 128 : (t + 1) * 128]
    wblk_host = wblk_host.astype(_bf16)
    S = tuple(int(v) for v in S_t)

    # ---- row vectors: bias-unit coeff and row-mask coeff per padded user ----
    rv = np.zeros((M, 2, UPAD), np.float32)
    rm = rowmask.reshape(M, UPC)
    rv[:, 0, :UPC] = (~rm).astype(np.float32)  # bias unit on for live rows
    rv[:, 1, :UPC] = rm.astype(np.float32)     # fill unit on for masked rows
    rv_host = rv.astype(_bf16)

    in_maps = []
    for c in range(M):
        in_maps.append(
            {
                "x": x_host[c],
                "encw": enc_host,
                "decw": dec_host,
                "sidx": si_host[c],
                "wblk": wblk_host[c],
                "rowvec": rv_host[c],
            }
        )
    return S, NE, in_maps


def _install_ntff_hook_shim():
    """The agent image's antenv lacks axon_hooks; synthesize it so
    run_bass_kernel_spmd(trace=True) can capture NTFF profiles."""
    import types

    if "antenv.axon_hooks" in sys.modules:
        return
    try:
        from trn_agent_boot.trn_boot import _ntff_profile_via_ctypes
    except ImportError:
        return
    hook = _ntff_profile_via_ctypes("/opt/axon/libaxon_pjrt.so")
    mod = types.ModuleType("antenv.axon_hooks")
    mod._hook = hook
    mod.set_axon_ntff_profile_hook = lambda h: setattr(mod, "_hook", h)
    mod.get_axon_ntff_profile_hook = lambda: mod._hook
    sys.modules["antenv.axon_hooks"] = mod
    try:
        import antenv

        antenv.axon_hooks = mod
    except ImportError:
        pass


LAST_EXEC_NS = None
LAST_RESULTS = None


def kernel(x, edge_index, edge_weight, ft_n0, ft_n1, fill_const,
           enc_w, enc_b, dec_w, dec_b, conv_w):
    global LAST_EXEC_NS, LAST_RESULTS
    from concourse.bass_utils import run_bass_kernel_spmd

    S, NE, in_maps = _prep_host(
        x, edge_index, edge_weight, ft_n0, ft_n1, fill_const,
        enc_w, enc_b, dec_w, dec_b, conv_w,
    )

    key = (S, NE)
    if key not in _PROGRAM_CACHE:
        _PROGRAM_CACHE[key] = _build_program(S, NE)
    nc = _PROGRAM_CACHE[key]

    trace = os.environ.get("KERNEL_TRACE", "0") == "1"
    tmpdir = os.environ.get("KERNEL_TRACE_DIR") or None
    if trace:
        _install_ntff_hook_shim()
    res = run_bass_kernel_spmd(
        nc,
        in_maps,
        core_ids=list(range(M)),
        trace=trace,
        tmpdir=tmpdir,
    )
    LAST_EXEC_NS = res.exec_time_ns
    LAST_RESULTS = res
    out = np.concatenate([res.results[c]["out"] for c in range(M)], axis=0)
    return np.ascontiguousarray(out.astype(np.float32))


# revision 23
# speedup vs baseline: 1.1194x; 1.1194x over previous
"""Trainium2 Bass kernel for nn_Autorec_DG_13116830122688 (AutoRec + GraphConv0D).

Math (reference):
    h   = sigmoid(x @ enc_w.T + enc_b)                      [N, 500]
    agg = segment_sum(h[src] * edge_weight, dst, N)
    hm  = conv_w * agg + (1 - conv_w) * h
    p   = clip(hm @ dec_w.T + dec_b, 1, 5)
    p   = where(ft_n0 == 0 rows, fill, p); where(ft_n1 == 0 cols, fill, p)

Strategy (8 NeuronCores, data-parallel over users):
  - Shard users 2500/core (padded to 2560 = 20x128 tiles).
  - x is transposed + cast to bf16 on the HOST into per-tile item-major
    blocks [20][128p][47k*128u], so the encoder is a pure matmul stream:
    one contiguous 1.5 MB DMA per user tile, 47 accumulating matmuls
    against SBUF-resident enc_w (bias folded in as an extra input col,
    hidden padded to 504 with zero weights so h cols 500-503 are defined
    -> no memset needed).  ACT sigmoid -> h bf16.
  - AllGather h (bf16) in 6 uneven chunks overlapped with the encoder;
    the last chunks are small so the exposed tail is short.  Gather
    blocks whose edges source only from the first 4 chunks ("early")
    read a slice of h_full, so they can start before the last
    all-gathers land; late blocks are host-packed to the tail of each
    tile's block list.
  - Message passing: edges filtered (masked-dst dropped), scaled by
    conv_w, sorted by dst, packed into 128-edge blocks per 128-dst tile.
    Gather h[src] via indirect DMA; multiply with host-built sparse
    [128e x 128d] weight blocks on the PE: aggT += G.T @ W in PSUM,
    hidden-major, feeding the decoder without a transpose.  All of a
    tile's weight blocks arrive in ONE packed DMA.  Self-loops use a
    plain DMA of this core's own h rows + a diagonal weight block.
  - Decoder: p = hmT.T @ dec_w.T with column mask / fill baked into
    host-prepped weights plus bias & row-fill hidden units.  DVE clips
    to [1,5] writing fp16; one 1.5 MB output DMA per tile (fp16 out is
    upcast to f32 on the host).
  - DMA queue split: x-stream + gather-side prefetches on the sync ring
    (no compute-dependent waits ahead of them), weights + h/out writes +
    rv on the scalar ring, indirect gathers + collectives on gpsimd.
  - Phase 3 is software-pipelined: tile t+1's message-passing matmuls +
    agg->hmT copy run in the middle of tile t's decoder, and gathers
    are issued two tiles ahead, so the PE never idles between tiles.
"""

import os
import sys

import numpy as np

for _p in ("/opt/trn_rl_repo",):
    if _p not in sys.path and os.path.isdir(_p):
        sys.path.insert(0, _p)

import ml_dtypes  # noqa: E402

# ---- problem constants (hardcoded per contest rules) ----
N_USERS = 20000
N_ITEMS = 6000
HIDDEN = 500
M = 8  # cores
UPC = N_USERS // M  # 2500 users per core
UT = 20  # user tiles per core
UPAD = UT * 128  # 2560
KC = 47  # item chunks of 128 (6016 = 47*128 >= 6001 incl. bias col)
IPAD = KC * 128  # 6016
HPAD = 504  # hidden padded: 4 chunks of 126 (500 real + bias/mask units)
R_MIN, R_MAX = 1.0, 5.0
# all-gather chunk boundaries in user tiles (cumulative).  The chunked
# all-gathers execute serially on the CC core and compete with the
# encoder for HBM, so the chain must start early (small first chunk) and
# end with small chunks (short exposed tail after the encoder).
CC_TILE_BOUNDS = [5, 10, 14, 17, 19, 20]
# sources in AG chunks 1..NEARLY_CHUNKS (user tiles < 17) are "early": their
# gather blocks only depend on the first NEARLY_CHUNKS all-gathers.
NEARLY_CHUNKS = 4

_bf16 = ml_dtypes.bfloat16
_f16 = np.float16

_PROGRAM_CACHE = {}


def _build_program(S, NE):
    """Build the SPMD Bass program. S = per-tile gather-block counts;
    NE = per-tile count of leading "early" blocks whose sources all live in
    AG chunks 1..NEARLY_CHUNKS."""
    import concourse.bass as bass
    import concourse.bacc as bacc
    import concourse.mybir as mybir
    from concourse.tile import TileContext

    P = 128
    f32 = mybir.dt.float32
    bf16 = mybir.dt.bfloat16
    f16 = mybir.dt.float16
    i32 = mybir.dt.int32
    NBLK = sum(S)
    SMAX = max(S)
    BOFF = [sum(S[:t]) for t in range(UT)]

    nc = bacc.Bacc(
        "TRN2",
        target_bir_lowering=False,
        debug=False,
        num_devices=M,
        num_swdge_queues=4,
    )

    x_d = nc.declare_dram_parameter("x", [UT, P, IPAD], bf16, isOutput=False)
    encw_d = nc.declare_dram_parameter("encw", [P, KC * HPAD], bf16, isOutput=False)
    decw_d = nc.declare_dram_parameter("decw", [P, 4 * N_ITEMS], bf16, isOutput=False)
    si_d = nc.declare_dram_parameter("sidx", [P, NBLK], i32, isOutput=False)
    wb_d = nc.declare_dram_parameter("wblk", [P, (NBLK + UT) * P], bf16, isOutput=False)
    rv_d = nc.declare_dram_parameter("rowvec", [2, UPAD], bf16, isOutput=False)
    out_d = nc.declare_dram_parameter("out", [UPC, N_ITEMS], f16, isOutput=True)

    h_loc = nc.dram_tensor("h_loc", [UPAD, HPAD], bf16)
    h_full = nc.dram_tensor("h_full", [M * UPAD, HPAD], bf16, addr_space="Shared")
    CC_BOUNDS = CC_TILE_BOUNDS

    with TileContext(nc) as tc:
        with (
            # tiles the PE streams from (moving + stationary sources that are
            # not DMA-hammered) live on the RIGHT side of SBUF; the DMA fire
            # hose (x stream, gathers, h/p staging) on the LEFT, to minimize
            # SBUF port contention with the PE's xbus reads.
            tc.tile_pool(name="const", bufs=1, side="right") as cpool,
            tc.tile_pool(name="hmt", bufs=2, side="right") as mpool,
            tc.tile_pool(name="wbl", bufs=3, side="right") as wpool,
            tc.tile_pool(name="xin", bufs=3, side="left") as xpool,
            tc.tile_pool(name="hsb", bufs=3, side="left") as hpool,
            tc.tile_pool(name="gat", bufs=2 * (SMAX + 1) + 2, side="left") as gpool,
            tc.tile_pool(name="pout", bufs=2, side="left") as opool,
            tc.tile_pool(name="ps_a", bufs=2, space="PSUM") as ps_a,
            tc.tile_pool(name="ps_d", bufs=6, space="PSUM") as ps_d,
        ):
            # weights on the scalar HWDGE ring, in pieces fine enough that
            # tile 0's matmul stream is never far behind the weight DMA.
            enc_sb = cpool.tile([P, KC * HPAD], bf16, tag="encw")
            for e0, e1 in ((0, 2), (2, 4), (4, 8), (8, 12), (12, 16), (16, 24),
                           (24, 32), (32, 40), (40, KC)):
                nc.scalar.dma_start(
                    out=enc_sb[:, e0 * HPAD : e1 * HPAD],
                    in_=encw_d[:, e0 * HPAD : e1 * HPAD],
                )
            si_sb = cpool.tile([P, NBLK], i32, tag="sidx")
            nc.scalar.dma_start(out=si_sb[:], in_=si_d[:])
            dec_sb = cpool.tile([P, 4 * N_ITEMS], bf16, tag="decw")
            nc.scalar.dma_start(out=dec_sb[:], in_=decw_d[:])

            # ---------------- Phase 1: encoder ----------------
            for ut in range(UT):
                xb = xpool.tile([P, IPAD], bf16, tag="xb")
                nc.sync.dma_start(out=xb[:], in_=x_d[ut])
                h_ps = ps_a.tile([P, 512], f32, tag="acc")
                for k in range(KC):
                    nc.tensor.matmul(
                        out=h_ps[:, :HPAD],
                        lhsT=xb[:, k * P : (k + 1) * P],
                        rhs=enc_sb[:, k * HPAD : (k + 1) * HPAD],
                        start=(k == 0),
                        stop=(k == KC - 1),
                    )
                hsb = hpool.tile([P, HPAD], bf16, tag="hsb")
                nc.scalar.activation(
                    out=hsb[:],
                    in_=h_ps[:, :HPAD],
                    func=mybir.ActivationFunctionType.Sigmoid,
                )
                nc.scalar.dma_start(
                    out=h_loc[ut * P : (ut + 1) * P, :], in_=hsb[:]
                )
                # ---- Phase 2 (interleaved): chunked all-gather ----
                if (ut + 1) in CC_BOUNDS:
                    j = CC_BOUNDS.index(ut + 1)
                    lo = 0 if j == 0 else CC_BOUNDS[j - 1]
                    hi = CC_BOUNDS[j]
                    off = M * lo * P
                    nc.gpsimd.collective_compute(
                        "AllGather",
                        mybir.AluOpType.bypass,
                        replica_groups=[list(range(M))],
                        ins=[h_loc[lo * P : hi * P, :]],
                        outs=[h_full[off : off + M * (hi - lo) * P, :]],
                    )

            # ---------------- Phase 3: message passing + decoder ----------------
            # Software-pipelined so the PE never idles (any PE stall resets a
            # ~3us half-speed ramp): gathers are issued two tiles ahead, and
            # tile t+1's message-passing matmuls + hmT copy run in the MIDDLE
            # of tile t's decoder so the agg->hmT copy latency is hidden.

            # blocks whose sources all land in AG chunks 1..NEARLY ("early"
            # blocks, host-packed first) can gather before the last all-
            # gathers complete: slicing h_full keeps the dependency off the
            # final AG chunks.
            EARLY_ROWS = M * CC_TILE_BOUNDS[NEARLY_CHUNKS - 1] * P

            def issue_early(t):
                gts = []
                for s in range(NE[t]):
                    b = BOFF[t] + s
                    gt = gpool.tile([P, HPAD], bf16, tag="gt", name=f"gt{t}_{s}")
                    nc.gpsimd.indirect_dma_start(
                        out=gt[:],
                        out_offset=None,
                        in_=h_full[0:EARLY_ROWS, :],
                        in_offset=bass.IndirectOffsetOnAxis(
                            ap=si_sb[:, b : b + 1], axis=0
                        ),
                    )
                    gts.append(gt)
                return gts

            def issue_late(t, gts):
                for s in range(NE[t], S[t]):
                    b = BOFF[t] + s
                    gt = gpool.tile([P, HPAD], bf16, tag="gt", name=f"gl{t}_{s}")
                    nc.gpsimd.indirect_dma_start(
                        out=gt[:],
                        out_offset=None,
                        in_=h_full[:],
                        in_offset=bass.IndirectOffsetOnAxis(
                            ap=si_sb[:, b : b + 1], axis=0
                        ),
                    )
                    gts.append(gt)
                # self-loop block: this core's own contiguous h rows via a
                # plain DMA and a diagonal weight block (no indirect gather).
                gself = gpool.tile([P, HPAD], bf16, tag="gt", name=f"gs{t}")
                nc.sync.dma_start(
                    out=gself[:], in_=h_loc[t * P : (t + 1) * P, :]
                )
                gts.append(gself)
                # all of this tile's weight blocks in one packed DMA
                nsb = S[t] + 1
                wsb = wpool.tile([P, (SMAX + 1) * P], bf16, tag="wb", name=f"wsb{t}")
                nc.sync.dma_start(
                    out=wsb[:, : nsb * P],
                    in_=wb_d[:, (BOFF[t] + t) * P : (BOFF[t] + t + nsb) * P],
                )
                return gts, wsb

            def issue_gathers(t):
                return issue_late(t, issue_early(t))

            def mp_compute(t, gts, wsb):
                nsb = S[t] + 1
                agg_ps = ps_a.tile([P, 512], f32, tag="acc", name=f"agg{t}")
                # keep each PSUM sub-region's accumulation group contiguous:
                # interleaved start=True matmuls in one bank clobber each
                # other's accumulation state.
                for c in range(4):
                    for s in range(nsb):
                        nc.tensor.matmul(
                            out=agg_ps[0:126, c * P : (c + 1) * P],
                            lhsT=gts[s][:, c * 126 : (c + 1) * 126],
                            rhs=wsb[:, s * P : (s + 1) * P],
                            start=(s == 0),
                            stop=(s == nsb - 1),
                        )
                hmT = mpool.tile([P, 512], bf16, tag="hmT", name=f"hmT{t}")
                nc.scalar.activation(
                    out=hmT[0:126, :],
                    in_=agg_ps[0:126, :],
                    func=mybir.ActivationFunctionType.Copy,
                )
                # hidden unit 500 (chunk 3, row 122): decoder-bias unit
                # hidden unit 501 (chunk 3, row 123): row-mask fill unit
                nc.scalar.dma_start(
                    out=hmT[122:124, 3 * P : 4 * P],
                    in_=rv_d[0:2, t * P : (t + 1) * P],
                )
                return hmT

            def dec_half(t, half, hmT, psb):
                # c-outer so 6 consecutive matmuls share the same stationary
                # operand (hmT chunk c) — saves the per-matmul weight-reload
                # chain when the compiler/hardware can exploit it.
                pps = []
                for nn in range(6):
                    pp = ps_d.tile(
                        [P, 512], f32, tag="pp", name=f"pp{t}_{half}_{nn}"
                    )
                    pps.append(pp)
                for c in range(4):
                    for nn in range(6):
                        n = half * 6 + nn
                        nc.tensor.matmul(
                            out=pps[nn][:, :500],
                            lhsT=hmT[0:126, c * P : (c + 1) * P],
                            rhs=dec_sb[0:126, c * N_ITEMS + n * 500 : c * N_ITEMS + (n + 1) * 500],
                            start=(c == 0),
                            stop=(c == 3),
                        )
                for nn in range(6):
                    n = half * 6 + nn
                    nc.vector.tensor_scalar(
                        out=psb[:, n * 500 : (n + 1) * 500],
                        in0=pps[nn][:, :500],
                        scalar1=R_MAX,
                        scalar2=R_MIN,
                        op0=mybir.AluOpType.min,
                        op1=mybir.AluOpType.max,
                    )

            # pipeline head: all early gathers for tiles 0/1 are issued before
            # any late gather so the gpsimd queue never stalls on the final
            # all-gathers while early work remains.
            e0 = issue_early(0)
            e1 = issue_early(1)
            pend = {0: issue_late(0, e0), 1: issue_late(1, e1)}
            hmTs = {0: mp_compute(0, *pend.pop(0))}
            for t in range(UT):
                hmT = hmTs.pop(t)
                psb = opool.tile([P, N_ITEMS], f16, tag="psb", name=f"psb{t}")
                nu = UPC - t * P if t == UT - 1 else P  # 68 on the last tile
                dec_half(t, 0, hmT, psb)
                if t + 2 < UT:
                    pend[t + 2] = issue_gathers(t + 2)
                if t + 1 < UT:
                    hmTs[t + 1] = mp_compute(t + 1, *pend.pop(t + 1))
                nc.scalar.dma_start(
                    out=out_d[t * P : t * P + nu, 0:3000],
                    in_=psb[:nu, 0:3000],
                )
                dec_half(t, 1, hmT, psb)
                nc.scalar.dma_start(
                    out=out_d[t * P : t * P + nu, 3000:6000],
                    in_=psb[:nu, 3000:6000],
                )

    nc.finalize()
    return nc


def _prep_host(x, edge_index, edge_weight, ft_n0, ft_n1, fill_const,
               enc_w, enc_b, dec_w, dec_b, conv_w):
    """All host-side preprocessing: sharding, weight prep, edge packing."""
    x = np.asarray(x, np.float32)
    src = np.asarray(edge_index[0], np.int64)
    dst = np.asarray(edge_index[1], np.int64)
    w = np.asarray(edge_weight, np.float32)
    ft_n0 = np.asarray(ft_n0)
    ft_n1 = np.asarray(ft_n1)
    fill = float(np.asarray(fill_const))
    conv = float(np.asarray(conv_w))
    enc_w = np.asarray(enc_w, np.float32)
    enc_b = np.asarray(enc_b, np.float32)
    dec_w = np.asarray(dec_w, np.float32)
    dec_b = np.asarray(dec_b, np.float32)

    rowmask = ft_n0 == 0  # rows forced to fill
    colmask = ft_n1 == 0  # cols forced to fill

    # ---- x per core: item-major per-tile blocks [UT, 128p, KC*128u] bf16 ----
    # x_host[c][t][p][k*128+j] = xpad[c, t*128+j, k*128+p]
    xp = np.zeros((M, UPAD, IPAD), np.float32)
    xp[:, :UPC, :N_ITEMS] = x.reshape(M, UPC, N_ITEMS)
    xp[:, :, N_ITEMS] = 1.0  # encoder-bias input column
    xpb = xp.astype(_bf16)
    del xp
    x_host = np.ascontiguousarray(
        xpb.reshape(M, UT, 128, KC, 128).transpose(0, 1, 4, 3, 2)
    ).reshape(M, UT, 128, IPAD)
    del xpb

    # ---- encoder weights: [6016, 504] -> [128, 47*504] chunk-major ----
    ewp = np.zeros((IPAD, HPAD), np.float32)
    ewp[:N_ITEMS, :HIDDEN] = enc_w.T
    ewp[N_ITEMS, :HIDDEN] = enc_b
    enc_host = np.ascontiguousarray(
        ewp.reshape(KC, 128, HPAD).transpose(1, 0, 2).reshape(128, KC * HPAD)
    ).astype(_bf16)

    # ---- decoder weights with baked column mask / bias / fill units ----
    dw = dec_w.T.copy()  # [500, 6000]
    dw[:, colmask] = 0.0
    hp = np.zeros((HPAD, N_ITEMS), np.float32)
    hp[:HIDDEN] = dw
    hp[HIDDEN] = np.where(colmask, fill, dec_b)  # bias unit
    hp[HIDDEN + 1] = fill  # row-mask fill unit (all cols)
    dec_host = np.zeros((128, 4, N_ITEMS), np.float32)
    dec_host[:126] = hp.reshape(4, 126, N_ITEMS).transpose(1, 0, 2)
    dec_host = np.ascontiguousarray(dec_host.reshape(128, 4 * N_ITEMS)).astype(_bf16)

    # ---- edges: filter masked dst, fold conv_w, sort by dst ----
    keep = ~rowmask[dst]
    src_a = src[keep]
    dst_a = dst[keep]
    w_a = w[keep] * conv

    order = np.argsort(dst_a, kind="stable")
    src_a, dst_a, w_a = src_a[order], dst_a[order], w_a[order]

    core = dst_a // UPC
    ldst = dst_a - core * UPC
    tile_g = core * UT + ldst // 128  # global tile id (sorted ascending)
    din = (ldst % 128).astype(np.int64)
    counts = np.bincount(tile_g, minlength=M * UT).reshape(M, UT)

    # gather index into the PADDED all-gathered h table.
    # h_full layout after the uneven chunked all-gather: chunk j covers local
    # rows [lo_j*128, hi_j*128) of every core, concatenated core-major:
    # row = off_j + core * crows_j + (local - lo_j*128)
    src_core = src_a // UPC
    src_loc = src_a % UPC
    bounds_rows = np.array([b * 128 for b in CC_TILE_BOUNDS])
    starts_rows = np.concatenate([[0], bounds_rows[:-1]])
    crows = bounds_rows - starts_rows
    offs = np.concatenate([[0], np.cumsum(M * crows)[:-1]])
    cjs = np.searchsorted(bounds_rows, src_loc, side="right")
    gsrc_e = (
        offs[cjs] + src_core * crows[cjs] + (src_loc - starts_rows[cjs])
    ).astype(np.int64)

    # per-TILE block quota (max over cores, so the SPMD program is identical
    # on every core) instead of one global maximum: ~15-20% fewer gathers.
    S_t = np.maximum(1, np.ceil(counts.max(axis=0) / 128).astype(np.int64))
    boff = np.concatenate([[0], np.cumsum(S_t)[:-1]])
    NBLK = int(S_t.sum())

    # per-tile packed layout: tile t's blocks at columns
    # [(boff[t]+t)*128, (boff[t]+t+S_t+1)*128); gathers first, self last.
    # Within each tile, edges whose source is in AG chunks 1..NEARLY_CHUNKS
    # ("early") are packed into the leading blocks; NE[t] = number of blocks
    # (uniform across cores) that contain only early edges.
    EARLY_LOC = CC_TILE_BOUNDS[NEARLY_CHUNKS - 1] * 128
    si_host = np.zeros((M, 128, NBLK), np.int32)
    wblk_host = np.zeros((M, 128, (NBLK + UT) * 128), np.float32)
    first_late_blk = np.full((M, UT), 10**6, np.int64)
    starts = np.zeros(M * UT + 1, np.int64)
    np.cumsum(counts.reshape(-1), out=starts[1:])
    for g in range(M * UT):
        c, t = divmod(g, UT)
        n = int(counts[c, t])
        sl = slice(starts[g], starts[g] + n)
        cap = int(S_t[t]) * 128
        late = src_loc[sl] >= EARLY_LOC
        order2 = np.argsort(late, kind="stable")  # early edges first
        gi = np.zeros(cap, np.int64)
        wi = np.zeros(cap, np.float32)
        di = np.zeros(cap, np.int64)
        gi[:n] = gsrc_e[sl][order2]
        wi[:n] = w_a[sl][order2]
        di[:n] = din[sl][order2]
        n_late = int(late.sum())
        first_late_blk[c, t] = (n - n_late) // 128 if n_late else int(S_t[t])
        b0 = int(boff[t])
        base = (b0 + t) * 128
        for q in range(int(S_t[t])):
            blk = slice(q * 128, (q + 1) * 128)
            si_host[c, :, b0 + q] = gi[blk]
            wblk_host[c, np.arange(128), base + q * 128 + di[blk]] = wi[blk]
    NE = tuple(
        int(min(first_late_blk[:, t].min(), S_t[t])) for t in range(UT)
    )
    # diagonal self-loop weight blocks at slot S_t[t] of each tile:
    # applies (1-conv)*live(d) to h_loc rows of tile t.
    lv = np.zeros((M, UPAD), np.float32)
    lv[:, :UPC] = (~rowmask).reshape(M, UPC).astype(np.float32) * (1.0 - conv)
    di128 = np.arange(128)
    for t in range(UT):
        base = (int(boff[t]) + t + int(S_t[t])) * 128
        wblk_host[:, di128, base + di128] = lv[:, t * 128 : (t + 1) * 128]
    wblk_host = wblk_host.astype(_bf16)
    S = tuple(int(v) for v in S_t)

    # ---- row vectors: bias-unit coeff and row-mask coeff per padded user ----
    rv = np.zeros((M, 2, UPAD), np.float32)
    rm = rowmask.reshape(M, UPC)
    rv[:, 0, :UPC] = (~rm).astype(np.float32)  # bias unit on for live rows
    rv[:, 1, :UPC] = rm.astype(np.float32)     # fill unit on for masked rows
    rv_host = rv.astype(_bf16)

    in_maps = []
    for c in range(M):
        in_maps.append(
            {
                "x": x_host[c],
                "encw": enc_host,
                "decw": dec_host,
                "sidx": si_host[c],
                "wblk": wblk_host[c],
                "rowvec": rv_host[c],
            }
        )
    return S, NE, in_maps


def _install_ntff_hook_shim():
    """The agent image's antenv lacks axon_hooks; synthesize it so
    run_bass_kernel_spmd(trace=True) can capture NTFF profiles."""
    import types

    if "antenv.axon_hooks" in sys.modules:
        return
    try:
        from trn_agent_boot.trn_boot import _ntff_profile_via_ctypes
    except ImportError:
        return
    hook = _ntff_profile_via_ctypes("/opt/axon/libaxon_pjrt.so")
    mod = types.ModuleType("antenv.axon_hooks")
    mod._hook = hook
    mod.set_axon_ntff_profile_hook = lambda h: setattr(mod, "_hook", h)
    mod.get_axon_ntff_profile_hook = lambda: mod._hook
    sys.modules["antenv.axon_hooks"] = mod
    try:
        import antenv

        antenv.axon_hooks = mod
    except ImportError:
        pass


LAST_EXEC_NS = None
LAST_RESULTS = None


def kernel(x, edge_index, edge_weight, ft_n0, ft_n1, fill_const,
           enc_w, enc_b, dec_w, dec_b, conv_w):
    global LAST_EXEC_NS, LAST_RESULTS
    from concourse.bass_utils import run_bass_kernel_spmd

    S, NE, in_maps = _prep_host(
        x, edge_index, edge_weight, ft_n0, ft_n1, fill_const,
        enc_w, enc_b, dec_w, dec_b, conv_w,
    )

    key = (S, NE)
    if key not in _PROGRAM_CACHE:
        _PROGRAM_CACHE[key] = _build_program(S, NE)
    nc = _PROGRAM_CACHE[key]

    trace = os.environ.get("KERNEL_TRACE", "0") == "1"
    tmpdir = os.environ.get("KERNEL_TRACE_DIR") or None
    if trace:
        _install_ntff_hook_shim()
    res = run_bass_kernel_spmd(
        nc,
        in_maps,
        core_ids=list(range(M)),
        trace=trace,
        tmpdir=tmpdir,
    )
    LAST_EXEC_NS = res.exec_time_ns
    LAST_RESULTS = res
    out = np.concatenate([res.results[c]["out"] for c in range(M)], axis=0)
    return np.ascontiguousarray(out.astype(np.float32))
